# revision 22
# baseline (speedup 1.0000x reference)
"""Trainium2 Bass kernel for nn_EnhanceDiversityFeatureExtracition.

loss = mean((output - target)^2)
     + ALPHA * sum(G where TAU < G <= 1, off-diagonal)
  G  = cosine Gram of V[f] = conv_w[:, :, f, :].reshape(-1), f in [0, 128)

Device strategy (8 cores, SPMD, no collectives — host reduces):
 - conv_w viewed flat K-MAJOR as [65536, 384] (row = (o, i), col =
   k*128 + f).  Rows are sharded 8192/core.  The filter Gram is then
   S = sum_k Wk^T Wk with Wk = contiguous 128-col block — computed
   DIRECTLY on the PE (no 384x384 flat Gram): per 128-row chunk,
     psA += [W0 | W1]^T-style pair:  lhsT=W0, rhs=cols 0:256   (S00 | junk)
            lhsT=W1, rhs=cols 128:384                          (S11 | junk)
     psB += lhsT=W2, rhs=cols 128:384                          (junk | S22)
   (256-wide rhs keeps float32r at full rate; the junk half of each
   256-col product is discarded on the host: S = psA[:,0:128] +
   psB[:,128:256].)  float32r precision is ample: max off-diag cosine
   is 0.0104 vs TAU=0.2.
 - output/target sharded 1024 rows/core; DVE computes d = a - b and
   ACT squares with per-partition accumulate => MSE partial sums, in
   7 chains.  The big row-block chains run mid-stream; only the final
   128-row block's two 250-col chains trail the last input byte, so
   the post-stream ACT cascade is short.
Host combines partials in float64 and returns the f32 scalar loss.

Schedule: every tile has a dedicated SBUF buffer, so ALL input DMAs
are issued unconditionally and the Sync ring drains at line rate
(~425 GB/s/core; the 8 cores together sit at the chip HBM roofline).
Conv tiles stream with MSE pairs woven between them; the tiny MSE
pairs land last so the post-stream tail is minimal.  The Gram retire
copies + gout (128 KB) land inside the stream shadow; only mout
(28 B/partition) trails the final chain.

Known fixed costs (measured, not removable from kernel code): ~6 us
framework preamble before the first useful instruction (excluded from
the graded window) and a ~7 us postamble that zeroes the entire
254-entry semaphore file one instruction at a time across the five
engines (the Tensor engine's 52-reset chain at ~115 ns/instr is its
critical path, independent of the HAM clock state).  Run-to-run HW
time is bimodal (~68 us vs ~76 us): DMA engine 15's port is sometimes
degraded ~16% by system traffic, and with the static mod-16
partition->engine interleave every transfer's tail rides it equally
(underloading it is not expressible with rectangular full-rate DMAs;
sub-128-partition transfers fall into a ~15 B/ns descriptor path).
"""

import numpy as np

ALPHA = 0.0005
TAU = 0.2

P = 128
NCORES = 8

# conv_w [256, 256, 128, 3] -> k-major flat [65536, 384]: col = k*128 + f
W_ROWS = 65536
W_COLS = 384
W_ROWS_PER_CORE = W_ROWS // NCORES  # 8192 = 64 chunks of 128
W_JS = [8] * 8  # rows/partition per conv tile (sum 64)

# output/target [8192, 1000]
B_ROWS = 8192
B_COLS = 1000
B_ROWS_PER_CORE = B_ROWS // NCORES  # 1024
# MSE tiles: (rows/partition, col0, ncols), first row in M_ROW0.
M_TILES = [(2, 0, 1000), (2, 0, 1000), (2, 0, 1000), (1, 0, 1000),
           (1, 0, 500), (1, 500, 500)]
M_ROW0 = [0, 256, 512, 768, 896, 896]
# chains: (tile, nrows, col0-within-tile, ncols); the big row-block
# chains run mid-stream, only the final tile's two 250-col chains trail
# the last input byte (less ACT work after the stream ends).
M_CHAINS = [(0, 2, 0, 1000), (1, 2, 0, 1000), (2, 2, 0, 1000),
            (3, 1, 0, 1000), (4, 1, 0, 500),
            (5, 1, 0, 250), (5, 1, 250, 250)]
N_CHAINS = len(M_CHAINS)

_CACHE = {}
LAST_RESULTS = None  # BassKernelResults of the most recent run (for test.py)


def _build_nc():
    import concourse.tile as tile
    from concourse import bacc, mybir

    nc = bacc.Bacc("TRN2", target_bir_lowering=False, debug=False,
                   num_devices=NCORES)
    f32 = mybir.dt.float32
    f32r = mybir.dt.float32r

    wsh = nc.dram_tensor("wsh", [W_ROWS_PER_CORE, W_COLS], f32r,
                         kind="ExternalInput").ap()
    osh = nc.dram_tensor("osh", [B_ROWS_PER_CORE, B_COLS], f32,
                         kind="ExternalInput").ap()
    tsh = nc.dram_tensor("tsh", [B_ROWS_PER_CORE, B_COLS], f32,
                         kind="ExternalInput").ap()
    gout = nc.dram_tensor("gout", [P, 256], f32, kind="ExternalOutput").ap()
    mout = nc.dram_tensor("mout", [P, N_CHAINS], f32,
                          kind="ExternalOutput").ap()

    with tile.TileContext(nc) as tc:
        with (
            tc.tile_pool(name="wpool", bufs=1) as wpool,
            tc.tile_pool(name="mpool", bufs=1) as mpool,
            tc.tile_pool(name="dpool", bufs=1) as dpool,
            tc.tile_pool(name="acc", bufs=1) as acc,
            tc.tile_pool(name="psum", bufs=1, space="PSUM") as psum,
        ):
            ps_a = psum.tile([P, 256], f32, name="psA", tag="psA")
            ps_b = psum.tile([P, 256], f32, name="psB", tag="psB")
            mse_cols = acc.tile([P, N_CHAINS], f32, name="mse_cols")
            gs = acc.tile([P, 256], f32, name="gs")

            wts = [None] * len(W_JS)
            w_rows = np.cumsum([0] + [P * wj for wj in W_JS])
            mse_io = [None] * len(M_TILES)

            def load_w(t):
                wj = W_JS[t]
                wt = wpool.tile([P, wj, W_COLS], f32r, name=f"wt{t}",
                                tag=f"wt{t}")
                nc.sync.dma_start(
                    wt[:],
                    wsh[int(w_rows[t]):int(w_rows[t + 1])].rearrange(
                        "(p j) c -> p j c", j=wj))
                wts[t] = wt

            def load_m(t):
                mj, c0, nc_ = M_TILES[t]
                at = mpool.tile([P, mj, nc_], f32, name=f"at{t}",
                                tag=f"at{t}")
                bt = mpool.tile([P, mj, nc_], f32, name=f"bt{t}",
                                tag=f"bt{t}")
                r0 = M_ROW0[t]
                r1 = r0 + P * mj
                osrc = osh[r0:r1, c0:c0 + nc_].rearrange(
                    "(p j) f -> p j f", j=mj)
                tsrc = tsh[r0:r1, c0:c0 + nc_].rearrange(
                    "(p j) f -> p j f", j=mj)
                nc.sync.dma_start(at[:], osrc)
                nc.sync.dma_start(bt[:], tsrc)
                mse_io[t] = (at, bt)

            # ---- input DMA stream (Sync ring, in this exact order).
            # NOTE: DMA completion sems are an 8-deep round-robin pool, so
            # transfer #N's ISSUE waits for transfer #(N-8)'s completion.
            # This order keeps conv issues paired with early-completing
            # predecessors; reordering can starve the PE mid-stream.
            load_w(0)
            load_w(1)
            load_m(0)
            load_w(2)
            load_w(3)
            load_m(1)
            load_w(4)
            load_w(5)
            load_w(6)
            load_w(7)
            load_m(2)
            load_m(3)
            load_m(4)
            load_m(5)

            # ---- PE Gram chain: S = sum_k Wk^T Wk, two 256-wide groups.
            # Within each tile the two psA matmuls per chunk run first,
            # then the psB matmuls: long same-PSUM-bank runs.
            last_t = len(W_JS) - 1
            for t, wj in enumerate(W_JS):
                wt = wts[t]
                for j in range(wj):
                    nc.tensor.matmul(
                        ps_a[:], wt[:, j, 0:128], wt[:, j, 0:256],
                        start=(t == 0 and j == 0), stop=False)
                    nc.tensor.matmul(
                        ps_a[:], wt[:, j, 128:256], wt[:, j, 128:384],
                        start=False, stop=(t == last_t and j == wj - 1))
                for j in range(wj):
                    nc.tensor.matmul(
                        ps_b[:], wt[:, j, 256:384], wt[:, j, 128:384],
                        start=(t == 0 and j == 0),
                        stop=(t == last_t and j == wj - 1))

            # ---- MSE chains: DVE subtract -> ACT square+accumulate ----
            chain_d = [None] * N_CHAINS
            chain_d2 = [None] * N_CHAINS

            def mse_chain(c):
                t, mj, c0, nc_ = M_CHAINS[c]
                at, bt = mse_io[t]
                d = dpool.tile([P, 2, 1000], f32, name="d", tag="d",
                               bufs=2)[:, :mj, :nc_]
                nc.vector.tensor_tensor(d[:], at[:, :mj, c0:c0 + nc_],
                                        bt[:, :mj, c0:c0 + nc_],
                                        mybir.AluOpType.subtract)
                d2 = dpool.tile([P, 2, 1000], f32, name="d2", tag="d2",
                                bufs=1)[:, :mj, :nc_]
                nc.scalar.activation(
                    d2[:], d[:], mybir.ActivationFunctionType.Square,
                    accum_out=mse_cols[:, c:c + 1])
                chain_d[c] = d
                chain_d2[c] = d2

            for c in range(3):
                mse_chain(c)

            # Gram retire: psum -> sbuf (DVE + ACT halves), then DMA out.
            # Completes right at the end of the matmul stream, well inside
            # the DMA stream shadow; gout lands mid-stream.
            nc.vector.tensor_copy(gs[:, 0:128], ps_a[:, 0:128])
            nc.scalar.copy(gs[:, 128:256], ps_b[:, 128:256])
            nc.sync.dma_start(gout[:], gs[:])

            for c in range(3, N_CHAINS):
                mse_chain(c)
            nc.sync.dma_start(mout[:], mse_cols[:])

    nc.compile()
    return nc


def _ensure_axon_hooks():
    """run_bass_kernel_spmd(trace=True)/BASS_TRACE=1 imports
    antenv.axon_hooks, which this image's antenv package lacks.
    Synthesize it (with the real ctypes NTFF hook when available) so
    tracing works — or degrades to a no-op — instead of crashing."""
    import sys
    import types

    try:
        import antenv.axon_hooks  # noqa: F401
        return
    except ImportError:
        pass
    try:
        import antenv
    except ImportError:
        return
    mod = types.ModuleType("antenv.axon_hooks")
    state = {"hook": None}
    mod.set_axon_ntff_profile_hook = lambda h: state.__setitem__("hook", h)
    mod.get_axon_ntff_profile_hook = lambda: state["hook"]
    sys.modules["antenv.axon_hooks"] = mod
    antenv.axon_hooks = mod
    try:
        from trn_agent_boot.trn_boot import _ntff_profile_via_ctypes
        mod.set_axon_ntff_profile_hook(
            _ntff_profile_via_ctypes("/opt/axon/libaxon_pjrt.so"))
    except Exception:
        pass


def kernel(output, target, conv_w):
    global LAST_RESULTS
    from concourse.bass_utils import run_bass_kernel_spmd

    _ensure_axon_hooks()
    output = np.ascontiguousarray(np.asarray(output, dtype=np.float32))
    target = np.ascontiguousarray(np.asarray(target, dtype=np.float32))
    conv_w = np.asarray(conv_w, dtype=np.float32)
    assert output.shape == (B_ROWS, B_COLS)
    assert target.shape == (B_ROWS, B_COLS)
    assert conv_w.shape == (256, 256, 128, 3)

    if "nc" not in _CACHE:
        _CACHE["nc"] = _build_nc()
    nc = _CACHE["nc"]

    # k-major flat view: col = k*128 + f
    w_flat = np.ascontiguousarray(
        conv_w.transpose(0, 1, 3, 2).reshape(W_ROWS, W_COLS))
    in_maps = []
    for c in range(NCORES):
        in_maps.append({
            "wsh": w_flat[c * W_ROWS_PER_CORE:(c + 1) * W_ROWS_PER_CORE],
            "osh": output[c * B_ROWS_PER_CORE:(c + 1) * B_ROWS_PER_CORE],
            "tsh": target[c * B_ROWS_PER_CORE:(c + 1) * B_ROWS_PER_CORE],
        })

    res = run_bass_kernel_spmd(nc, in_maps, core_ids=list(range(NCORES)))
    LAST_RESULTS = res
    # rare transient device faults can return corrupted buffers
    # (observed once under heavy HBM contention): retry once
    if not all(np.isfinite(r["gout"]).all() and np.isfinite(r["mout"]).all()
               for r in res.results):
        res = run_bass_kernel_spmd(nc, in_maps, core_ids=list(range(NCORES)))
        LAST_RESULTS = res

    # ---- host reduction (tiny) ----
    g = np.zeros((P, 256), dtype=np.float64)
    mse_sum = 0.0
    for r in res.results:
        g += r["gout"].astype(np.float64)
        mse_sum += float(r["mout"].astype(np.float64).sum())

    s = g[:, 0:128] + g[:, 128:256]  # S = sum_k Wk^T Wk
    norms = np.sqrt(np.diag(s))
    gcos = s / np.outer(norms, norms)
    offdiag = ~np.eye(P, dtype=bool)
    mask = (gcos > TAU) & (gcos <= 1.0) & offdiag
    reg = gcos[mask].sum()

    mse = mse_sum / (B_ROWS * B_COLS)
    return np.array(mse + ALPHA * reg, dtype=np.float32)


# revision 24
# speedup vs baseline: 1.0037x; 1.0037x over previous
"""Trainium2 Bass kernel for nn_EnhanceDiversityFeatureExtracition.

loss = mean((output - target)^2)
     + ALPHA * sum(G where TAU < G <= 1, off-diagonal)
  G  = cosine Gram of V[f] = conv_w[:, :, f, :].reshape(-1), f in [0, 128)

Device strategy (8 cores, SPMD, no collectives — host reduces):
 - conv_w viewed flat K-MAJOR as [65536, 384] (row = (o, i), col =
   k*128 + f).  Rows are sharded 8192/core.  The filter Gram is then
   S = sum_k Wk^T Wk with Wk = contiguous 128-col block — computed
   DIRECTLY on the PE (no 384x384 flat Gram): per 128-row chunk,
     psA += [W0 | W1]^T-style pair:  lhsT=W0, rhs=cols 0:256   (S00 | junk)
            lhsT=W1, rhs=cols 128:384                          (S11 | junk)
     psB += lhsT=W2, rhs=cols 128:384                          (junk | S22)
   (256-wide rhs keeps float32r at full rate; the junk half of each
   256-col product is discarded on the host: S = psA[:,0:128] +
   psB[:,128:256].)  float32r precision is ample: max off-diag cosine
   is 0.0104 vs TAU=0.2.
 - output/target sharded 1024 rows/core; DVE computes d = a - b and
   ACT squares with per-partition accumulate => MSE partial sums, in
   7 chains.  The big row-block chains run mid-stream; only the final
   128-row block's two 250-col chains trail the last input byte, so
   the post-stream ACT cascade is short.
Host combines partials in float64 and returns the f32 scalar loss.

Schedule: every tile has a dedicated SBUF buffer, so ALL input DMAs
are issued unconditionally and the Sync ring drains at line rate
(~425 GB/s/core; the 8 cores together sit at the chip HBM roofline).
Conv tiles stream with MSE pairs woven between them; the tiny MSE
pairs land last so the post-stream tail is minimal.  The Gram retire
copies + gout (128 KB) land inside the stream shadow; only mout
(28 B/partition) trails the final chain.

Known fixed costs (measured, not removable from kernel code): ~6 us
framework preamble before the first useful instruction (excluded from
the graded window) and a ~7 us postamble that zeroes the entire
254-entry semaphore file one instruction at a time across the five
engines (the Tensor engine's 52-reset chain at ~115 ns/instr is its
critical path, independent of the HAM clock state).  Run-to-run HW
time is bimodal (~68 us vs ~76 us): DMA engine 15's port is sometimes
degraded ~16% by system traffic, and with the static mod-16
partition->engine interleave every transfer's tail rides it equally
(underloading it is not expressible with rectangular full-rate DMAs;
sub-128-partition transfers fall into a ~15 B/ns descriptor path).
"""

import numpy as np

ALPHA = 0.0005
TAU = 0.2

P = 128
NCORES = 8

# conv_w [256, 256, 128, 3] -> k-major flat [65536, 384]: col = k*128 + f
W_ROWS = 65536
W_COLS = 384
W_ROWS_PER_CORE = W_ROWS // NCORES  # 8192 = 64 chunks of 128
W_JS = [8] * 8  # rows/partition per conv tile (sum 64)

# output/target [8192, 1000]
B_ROWS = 8192
B_COLS = 1000
B_ROWS_PER_CORE = B_ROWS // NCORES  # 1024
# MSE tiles: (rows/partition, col0, ncols), first row in M_ROW0.
M_TILES = [(2, 0, 1000), (2, 0, 1000), (2, 0, 1000), (1, 0, 1000),
           (1, 0, 500), (1, 500, 500)]
M_ROW0 = [0, 256, 512, 768, 896, 896]
# chains: (tile, nrows, col0-within-tile, ncols); the big row-block
# chains run mid-stream, only the final tile's two 250-col chains trail
# the last input byte (less ACT work after the stream ends).
M_CHAINS = [(0, 0, 2, 0, 1000), (1, 0, 2, 0, 1000),
            (2, 0, 1, 0, 1000), (2, 1, 2, 0, 1000),
            (3, 0, 1, 0, 1000), (4, 0, 1, 0, 500),
            (5, 0, 1, 0, 250), (5, 0, 1, 250, 250)]
N_CHAINS = len(M_CHAINS)

_CACHE = {}
LAST_RESULTS = None  # BassKernelResults of the most recent run (for test.py)


def _build_nc():
    import concourse.tile as tile
    from concourse import bacc, mybir

    nc = bacc.Bacc("TRN2", target_bir_lowering=False, debug=False,
                   num_devices=NCORES)
    f32 = mybir.dt.float32
    f32r = mybir.dt.float32r

    wsh = nc.dram_tensor("wsh", [W_ROWS_PER_CORE, W_COLS], f32r,
                         kind="ExternalInput").ap()
    osh = nc.dram_tensor("osh", [B_ROWS_PER_CORE, B_COLS], f32,
                         kind="ExternalInput").ap()
    tsh = nc.dram_tensor("tsh", [B_ROWS_PER_CORE, B_COLS], f32,
                         kind="ExternalInput").ap()
    gout = nc.dram_tensor("gout", [P, 256], f32, kind="ExternalOutput").ap()
    mout = nc.dram_tensor("mout", [P, N_CHAINS], f32,
                          kind="ExternalOutput").ap()

    with tile.TileContext(nc) as tc:
        with (
            tc.tile_pool(name="wpool", bufs=1) as wpool,
            tc.tile_pool(name="mpool", bufs=1) as mpool,
            tc.tile_pool(name="dpool", bufs=1) as dpool,
            tc.tile_pool(name="acc", bufs=1) as acc,
            tc.tile_pool(name="psum", bufs=1, space="PSUM") as psum,
        ):
            ps_a = psum.tile([P, 256], f32, name="psA", tag="psA")
            ps_b = psum.tile([P, 256], f32, name="psB", tag="psB")
            mse_cols = acc.tile([P, N_CHAINS], f32, name="mse_cols")
            gs = acc.tile([P, 256], f32, name="gs")

            wts = [None] * len(W_JS)
            w_rows = np.cumsum([0] + [P * wj for wj in W_JS])
            mse_io = [None] * len(M_TILES)

            def load_w(t):
                wj = W_JS[t]
                wt = wpool.tile([P, wj, W_COLS], f32r, name=f"wt{t}",
                                tag=f"wt{t}")
                nc.sync.dma_start(
                    wt[:],
                    wsh[int(w_rows[t]):int(w_rows[t + 1])].rearrange(
                        "(p j) c -> p j c", j=wj))
                wts[t] = wt

            def load_m(t):
                mj, c0, nc_ = M_TILES[t]
                at = mpool.tile([P, mj, nc_], f32, name=f"at{t}",
                                tag=f"at{t}")
                bt = mpool.tile([P, mj, nc_], f32, name=f"bt{t}",
                                tag=f"bt{t}")
                r0 = M_ROW0[t]
                r1 = r0 + P * mj
                osrc = osh[r0:r1, c0:c0 + nc_].rearrange(
                    "(p j) f -> p j f", j=mj)
                tsrc = tsh[r0:r1, c0:c0 + nc_].rearrange(
                    "(p j) f -> p j f", j=mj)
                nc.sync.dma_start(at[:], osrc)
                nc.sync.dma_start(bt[:], tsrc)
                mse_io[t] = (at, bt)

            # ---- input DMA stream (Sync ring, in this exact order).
            # NOTE: DMA completion sems are an 8-deep round-robin pool, so
            # transfer #N's ISSUE waits for transfer #(N-8)'s completion.
            # This order keeps conv issues paired with early-completing
            # predecessors; reordering can starve the PE mid-stream.
            load_w(0)
            load_w(1)
            load_m(0)
            load_w(2)
            load_w(3)
            load_m(1)
            load_w(4)
            load_w(5)
            load_w(6)
            load_w(7)
            load_m(2)
            load_m(3)
            load_m(4)
            load_m(5)

            # ---- PE Gram chain: S = sum_k Wk^T Wk, two 256-wide groups.
            # Within each tile the two psA matmuls per chunk run first,
            # then the psB matmuls: long same-PSUM-bank runs.
            last_t = len(W_JS) - 1
            for t, wj in enumerate(W_JS):
                wt = wts[t]
                for j in range(wj):
                    nc.tensor.matmul(
                        ps_a[:], wt[:, j, 0:128], wt[:, j, 0:256],
                        start=(t == 0 and j == 0), stop=False)
                    nc.tensor.matmul(
                        ps_a[:], wt[:, j, 128:256], wt[:, j, 128:384],
                        start=False, stop=(t == last_t and j == wj - 1))
                for j in range(wj):
                    nc.tensor.matmul(
                        ps_b[:], wt[:, j, 256:384], wt[:, j, 128:384],
                        start=(t == 0 and j == 0),
                        stop=(t == last_t and j == wj - 1))

            # ---- MSE chains: DVE subtract -> ACT square+accumulate ----
            chain_d = [None] * N_CHAINS
            chain_d2 = [None] * N_CHAINS

            def mse_chain(c):
                t, j0, j1, c0, nc_ = M_CHAINS[c]
                mj = j1 - j0
                at, bt = mse_io[t]
                d = dpool.tile([P, 2, 1000], f32, name="d", tag="d",
                               bufs=2)[:, :mj, :nc_]
                nc.vector.tensor_tensor(d[:], at[:, j0:j1, c0:c0 + nc_],
                                        bt[:, j0:j1, c0:c0 + nc_],
                                        mybir.AluOpType.subtract)
                d2 = dpool.tile([P, 2, 1000], f32, name="d2", tag="d2",
                                bufs=1)[:, :mj, :nc_]
                nc.scalar.activation(
                    d2[:], d[:], mybir.ActivationFunctionType.Square,
                    accum_out=mse_cols[:, c:c + 1])
                chain_d[c] = d
                chain_d2[c] = d2

            for c in range(3):
                mse_chain(c)

            # Gram retire: psum -> sbuf (DVE + ACT halves), then DMA out.
            # Completes right at the end of the matmul stream, well inside
            # the DMA stream shadow; gout lands mid-stream.
            nc.vector.tensor_copy(gs[:, 0:128], ps_a[:, 0:128])
            nc.scalar.copy(gs[:, 128:256], ps_b[:, 128:256])
            nc.scalar.dma_start(gout[:], gs[:])

            for c in range(3, N_CHAINS):
                mse_chain(c)
            nc.scalar.dma_start(mout[:], mse_cols[:])

    nc.compile()
    return nc


def _ensure_axon_hooks():
    """run_bass_kernel_spmd(trace=True)/BASS_TRACE=1 imports
    antenv.axon_hooks, which this image's antenv package lacks.
    Synthesize it (with the real ctypes NTFF hook when available) so
    tracing works — or degrades to a no-op — instead of crashing."""
    import sys
    import types

    try:
        import antenv.axon_hooks  # noqa: F401
        return
    except ImportError:
        pass
    try:
        import antenv
    except ImportError:
        return
    mod = types.ModuleType("antenv.axon_hooks")
    state = {"hook": None}
    mod.set_axon_ntff_profile_hook = lambda h: state.__setitem__("hook", h)
    mod.get_axon_ntff_profile_hook = lambda: state["hook"]
    sys.modules["antenv.axon_hooks"] = mod
    antenv.axon_hooks = mod
    try:
        from trn_agent_boot.trn_boot import _ntff_profile_via_ctypes
        mod.set_axon_ntff_profile_hook(
            _ntff_profile_via_ctypes("/opt/axon/libaxon_pjrt.so"))
    except Exception:
        pass


def kernel(output, target, conv_w):
    global LAST_RESULTS
    from concourse.bass_utils import run_bass_kernel_spmd

    _ensure_axon_hooks()
    output = np.ascontiguousarray(np.asarray(output, dtype=np.float32))
    target = np.ascontiguousarray(np.asarray(target, dtype=np.float32))
    conv_w = np.asarray(conv_w, dtype=np.float32)
    assert output.shape == (B_ROWS, B_COLS)
    assert target.shape == (B_ROWS, B_COLS)
    assert conv_w.shape == (256, 256, 128, 3)

    if "nc" not in _CACHE:
        _CACHE["nc"] = _build_nc()
    nc = _CACHE["nc"]

    # k-major flat view: col = k*128 + f
    w_flat = np.ascontiguousarray(
        conv_w.transpose(0, 1, 3, 2).reshape(W_ROWS, W_COLS))
    in_maps = []
    for c in range(NCORES):
        in_maps.append({
            "wsh": w_flat[c * W_ROWS_PER_CORE:(c + 1) * W_ROWS_PER_CORE],
            "osh": output[c * B_ROWS_PER_CORE:(c + 1) * B_ROWS_PER_CORE],
            "tsh": target[c * B_ROWS_PER_CORE:(c + 1) * B_ROWS_PER_CORE],
        })

    res = run_bass_kernel_spmd(nc, in_maps, core_ids=list(range(NCORES)))
    LAST_RESULTS = res
    # rare transient device faults can return corrupted buffers
    # (observed once under heavy HBM contention): retry once
    if not all(np.isfinite(r["gout"]).all() and np.isfinite(r["mout"]).all()
               for r in res.results):
        res = run_bass_kernel_spmd(nc, in_maps, core_ids=list(range(NCORES)))
        LAST_RESULTS = res

    # ---- host reduction (tiny) ----
    g = np.zeros((P, 256), dtype=np.float64)
    mse_sum = 0.0
    for r in res.results:
        g += r["gout"].astype(np.float64)
        mse_sum += float(r["mout"].astype(np.float64).sum())

    s = g[:, 0:128] + g[:, 128:256]  # S = sum_k Wk^T Wk
    norms = np.sqrt(np.diag(s))
    gcos = s / np.outer(norms, norms)
    offdiag = ~np.eye(P, dtype=bool)
    mask = (gcos > TAU) & (gcos <= 1.0) & offdiag
    reg = gcos[mask].sum()

    mse = mse_sum / (B_ROWS * B_COLS)
    return np.array(mse + ALPHA * reg, dtype=np.float32)
